# revision 6
# baseline (speedup 1.0000x reference)
"""BiRNN encoder-decoder Trainium2 kernel.

Strategy: data-parallel over batch (8 cores x 16 rows). All matmuls in
float32r (tf32-like, 1 cyc/row at moving>=256). Recurrences use "layout B":
psum out = (b, H) with state-transpose (hT) as the stationary operand,
weights as the 512-col moving operand; hT is regenerated each step via 4
PE-transposes + one DVE copy. Layer-0 input projections (+all biases) are
precomputed on the host and DMA-prefetched per step; layer-1's input
projection is fused into its recurrence as 8 extra moving-weight matmuls
against the stored layer-0 hT chunks.
"""
import numpy as np
from contextlib import ExitStack

import concourse.bacc as bacc
import concourse.tile as tile
from concourse import mybir
from concourse.bass_utils import run_bass_kernel_spmd

B, T, IN, H, TGT = 128, 128, 3, 512, 32
NC = 8
BC = B // NC  # 16 batch rows per core
F32R = mybir.dt.float32r
F32 = mybir.dt.float32
CH = H // 128  # 4 chunks of the hidden dim

_prog_cache = {}


def _build_program():
    if "nc" in _prog_cache:
        return _prog_cache["nc"]
    nc = bacc.Bacc("TRN2")
    dp = nc.declare_dram_parameter

    xs0_e = dp("xs0", [2, T, BC, H], F32R, isOutput=False)          # per-dir l0 x-proj (+biases), bwd time-reversed
    whh0_e = dp("whh0", [2, CH, 128, H], F32R, isOutput=False)      # enc l0 Whh.T chunks
    whh1_e = dp("whh1", [2, CH, 128, H], F32R, isOutput=False)      # enc l1 Whh.T chunks
    wih1_e = dp("wih1", [2, 2 * CH, 128, H], F32R, isOutput=False)  # enc l1 Wih.T chunks (k<4 fwd-half, k>=4 bwd-half)
    bias1_e = dp("bias1", [2, 1, H], F32R, isOutput=False)          # enc l1 bih+bhh rows
    dwhh_e = dp("dwhh", [4, CH, 128, H], F32R, isOutput=False)      # dec Whh.T chunks
    dwihr_e = dp("dwihr", [3, CH, 128, H], F32R, isOutput=False)    # dec Wihr.T chunks
    dbias_e = dp("dbias", [3, 1, H], F32R, isOutput=False)          # dec l1..3 bias rows
    din0w_e = dp("din0w", [16, H], F32R, isOutput=False)             # [dec_Wih0.T(3,H); bias0(1,H)]
    linw_e = dp("linw", [CH, 128, 16], F32R, isOutput=False)         # lin_W.T chunks
    linb_e = dp("linb", [1, 16], F32R, isOutput=False)
    ident_e = dp("ident", [16, 16], F32R, isOutput=False)
    ones1_e = dp("ones1", [1, 16], F32R, isOutput=False)
    dinit_e = dp("dinit", [BC, 16], F32R, isOutput=False)            # [x_last(16,3), ones]
    out_e = dp("out", [BC, TGT], F32, isOutput=True)

    with tile.TileContext(nc) as tc, ExitStack() as ctx:
        wpool = ctx.enter_context(tc.tile_pool(name="w", bufs=1))
        hts = ctx.enter_context(tc.tile_pool(name="hts", bufs=1))
        sbuf = ctx.enter_context(tc.tile_pool(name="sb", bufs=1))
        pspool = ctx.enter_context(tc.tile_pool(name="ps", bufs=1, space="PSUM"))

        # ---- weights / constants into SBUF ----
        whh0 = wpool.tile([128, 2 * CH * H], F32R)
        whh1 = wpool.tile([128, 2 * CH * H], F32R)
        wih1 = wpool.tile([128, 2 * 2 * CH * H], F32R)
        dwhh = wpool.tile([128, 4 * CH * H], F32R)
        dwihr = wpool.tile([128, 3 * CH * H], F32R)
        for d in range(2):
            for c in range(CH):
                nc.gpsimd.dma_start(whh0[:, (d * CH + c) * H:(d * CH + c + 1) * H], whh0_e[d, c])
                nc.gpsimd.dma_start(whh1[:, (d * CH + c) * H:(d * CH + c + 1) * H], whh1_e[d, c])
            for k in range(2 * CH):
                nc.gpsimd.dma_start(wih1[:, (d * 2 * CH + k) * H:(d * 2 * CH + k + 1) * H], wih1_e[d, k])
        for l in range(4):
            for c in range(CH):
                nc.gpsimd.dma_start(dwhh[:, (l * CH + c) * H:(l * CH + c + 1) * H], dwhh_e[l, c])
        for l in range(3):
            for c in range(CH):
                nc.gpsimd.dma_start(dwihr[:, (l * CH + c) * H:(l * CH + c + 1) * H], dwihr_e[l, c])
        linw = wpool.tile([128, CH * 16], F32R)
        for c in range(CH):
            nc.gpsimd.dma_start(linw[:, 16 * c:16 * (c + 1)], linw_e[c])
        bias1 = wpool.tile([1, 2 * H], F32R)
        dbias = wpool.tile([1, 3 * H], F32R)
        for d in range(2):
            nc.gpsimd.dma_start(bias1[:, d * H:(d + 1) * H], bias1_e[d])
        for l in range(3):
            nc.gpsimd.dma_start(dbias[:, l * H:(l + 1) * H], dbias_e[l])
        din0w = wpool.tile([16, H], F32R)
        nc.gpsimd.dma_start(din0w[:], din0w_e[:])
        linb = wpool.tile([1, 16], F32R)
        nc.gpsimd.dma_start(linb[:], linb_e[:])
        ident = wpool.tile([16, 16], F32R)
        nc.gpsimd.dma_start(ident[:], ident_e[:])
        ones1 = wpool.tile([1, 16], F32R)
        nc.gpsimd.dma_start(ones1[:], ones1_e[:])
        onescol = wpool.tile([BC, 13], F32R)
        nc.gpsimd.dma_start(onescol[:], dinit_e[:, 3:16])

        # per-t l0 hidden-state transposes, both directions (bwd in its own step order)
        hT0 = {d: hts.tile([128, T * 4 * BC], F32R, tag=f"hT0_{d}", name=f"hT0_{d}") for d in range(2)}

        def recur_step(ps_tag, h_tag, whh_tile, whh_off, hT_cur, hT_dst, inject):
            """One RNN step: psum = inject + h @ Whh.T; tanh; rebuild hT."""
            ps = pspool.tile([BC, H], F32, tag=ps_tag, name=ps_tag, bufs=2)
            inject(ps)
            for c in range(CH):
                nc.tensor.matmul(ps[:], hT_cur[:, 16 * c:16 * (c + 1)],
                                 whh_tile[:, whh_off + c * H: whh_off + (c + 1) * H],
                                 start=False, stop=(c == CH - 1))
            h = sbuf.tile([BC, H], F32R, tag="h", name="h", bufs=3)
            nc.scalar.activation(h[:], ps[:], mybir.ActivationFunctionType.Tanh)
            psT = pspool.tile([128, 4 * BC], F32R, tag="psT", name="psT", bufs=2)
            for c in range(CH):
                nc.tensor.transpose(psT[:, 16 * c:16 * (c + 1)], h[:, 128 * c:128 * (c + 1)], ident[:])
            nc.vector.tensor_copy(hT_dst[:], psT[:])
            return h

        # ---- encoder layer 0 (fwd chain d=0, bwd chain d=1, interleaved) ----
        hT_cur = {}
        for t in range(T):
            for d in range(2):
                xs = sbuf.tile([BC, H], F32R, tag="xs", name="xs")
                nc.sync.dma_start(xs[:], xs0_e[d, t])

                def inject(ps, xs=xs):
                    nc.tensor.matmul(ps[:], ident[:], xs[:], start=True, stop=False)
                dst = hT0[d][:, t * 4 * BC:(t + 1) * 4 * BC]
                if t == 0:
                    # h0 = 0: first Whh matmul contributes zeros via zeroed hT? No -
                    # instead: psum = inject only, skip Whh matmuls entirely.
                    ps = pspool.tile([BC, H], F32, tag="psA" if d == 0 else "psB", name="ps0", bufs=2)
                    nc.tensor.matmul(ps[:], ident[:], xs[:], start=True, stop=True)
                    h = sbuf.tile([BC, H], F32R, tag="h", name="h", bufs=3)
                    nc.scalar.activation(h[:], ps[:], mybir.ActivationFunctionType.Tanh)
                    psT = pspool.tile([128, 4 * BC], F32R, tag="psT", name="psT", bufs=2)
                    for c in range(CH):
                        nc.tensor.transpose(psT[:, 16 * c:16 * (c + 1)], h[:, 128 * c:128 * (c + 1)], ident[:])
                    nc.vector.tensor_copy(dst, psT[:])
                else:
                    recur_step("psA" if d == 0 else "psB", f"h0_{d}", whh0, d * CH * H, hT_cur[d], dst, inject)
                hT_cur[d] = dst

        # ---- encoder layer 1 (input = stored l0 hT of both dirs, fused proj) ----
        hT1 = {}
        for t in range(T):
            for d in range(2):
                # kernel-step t of chain d corresponds to real time rt:
                # d=0: rt=t -> fwd l0 slot t, bwd l0 slot T-1-t
                # d=1: rt=T-1-t -> fwd l0 slot T-1-t, bwd l0 slot t
                f_slot = t if d == 0 else T - 1 - t
                b_slot = T - 1 - t if d == 0 else t

                def inject(ps, d=d, f_slot=f_slot, b_slot=b_slot):
                    nc.tensor.matmul(ps[:], ones1[:], bias1[:, d * H:(d + 1) * H], start=True, stop=False)
                    for k in range(2 * CH):
                        src = hT0[0] if k < CH else hT0[1]
                        slot = f_slot if k < CH else b_slot
                        cc = k % CH
                        nc.tensor.matmul(
                            ps[:], src[:, slot * 4 * BC + 16 * cc: slot * 4 * BC + 16 * (cc + 1)],
                            wih1[:, (d * 2 * CH + k) * H:(d * 2 * CH + k + 1) * H],
                            start=False, stop=False)
                dst = hts.tile([128, 4 * BC], F32R, tag=f"hT1_{d}", name=f"hT1_{d}", bufs=2)
                if t == 0:
                    # no Whh term at t=0 (h0 = 0): close the group on the last inject mm
                    ps = pspool.tile([BC, H], F32, tag="psA" if d == 0 else "psB", name="ps1", bufs=2)
                    nc.tensor.matmul(ps[:], ones1[:], bias1[:, d * H:(d + 1) * H], start=True, stop=False)
                    for k in range(2 * CH):
                        src = hT0[0] if k < CH else hT0[1]
                        slot = f_slot if k < CH else b_slot
                        cc = k % CH
                        nc.tensor.matmul(
                            ps[:], src[:, slot * 4 * BC + 16 * cc: slot * 4 * BC + 16 * (cc + 1)],
                            wih1[:, (d * 2 * CH + k) * H:(d * 2 * CH + k + 1) * H],
                            start=False, stop=(k == 2 * CH - 1))
                    h = sbuf.tile([BC, H], F32R, tag="h", name="h", bufs=3)
                    nc.scalar.activation(h[:], ps[:], mybir.ActivationFunctionType.Tanh)
                    psT = pspool.tile([128, 4 * BC], F32R, tag="psT", name="psT", bufs=2)
                    for c in range(CH):
                        nc.tensor.transpose(psT[:, 16 * c:16 * (c + 1)], h[:, 128 * c:128 * (c + 1)], ident[:])
                    nc.vector.tensor_copy(dst[:], psT[:])
                else:
                    recur_step("psA" if d == 0 else "psB", f"h1_{d}", whh1, d * CH * H, hT1[d], dst[:], inject)
                hT1[d] = dst[:]

        # ---- decoder: 4-layer stack, 32 autoregressive steps ----
        hT_dec = {0: hT0[0][:, (T - 1) * 4 * BC: T * 4 * BC],  # hf0
                  1: hT0[1][:, (T - 1) * 4 * BC: T * 4 * BC],  # hb0 (its last kernel step = real t0)
                  2: hT1[0], 3: hT1[1]}
        xin = sbuf.tile([BC, 16], F32R, tag="xin", name="xin", bufs=2)
        nc.sync.dma_start(xin[:], dinit_e[:])
        outcol = sbuf.tile([BC, TGT], F32, tag="outcol", name="outcol")

        for t in range(TGT):
            # transpose current input cols (16,4) -> (4,16) for the l0 inject
            psx = pspool.tile([16, 16], F32R, tag="psS", name="psxT", bufs=2)
            nc.tensor.transpose(psx[:], xin[:], ident[:])
            xinT = sbuf.tile([16, 16], F32R, tag="xinT", name="xinT")
            nc.vector.tensor_copy(xinT[:], psx[:])

            h_below = None
            for l in range(4):
                ps = pspool.tile([BC, H], F32, tag="psA", name="ps_dec", bufs=2)
                if l == 0:
                    nc.tensor.matmul(ps[:], xinT[:], din0w[:], start=True, stop=False)
                else:
                    nc.tensor.matmul(ps[:], ones1[:], dbias[:, (l - 1) * H:l * H], start=True, stop=False)
                    for c in range(CH):
                        nc.tensor.matmul(ps[:], h_below[:, 16 * c:16 * (c + 1)],
                                         dwihr[:, ((l - 1) * CH + c) * H:((l - 1) * CH + c + 1) * H],
                                         start=False, stop=False)
                for c in range(CH):
                    nc.tensor.matmul(ps[:], hT_dec[l][:, 16 * c:16 * (c + 1)],
                                     dwhh[:, (l * CH + c) * H:(l * CH + c + 1) * H],
                                     start=False, stop=(c == CH - 1))
                h = sbuf.tile([BC, H], F32R, tag="h", name="h", bufs=3)
                nc.scalar.activation(h[:], ps[:], mybir.ActivationFunctionType.Tanh)
                psT = pspool.tile([128, 4 * BC], F32R, tag="psT", name="psT", bufs=2)
                for c in range(CH):
                    nc.tensor.transpose(psT[:, 16 * c:16 * (c + 1)], h[:, 128 * c:128 * (c + 1)], ident[:])
                hT_new = hts.tile([128, 4 * BC], F32R, tag=f"hTd_{l}", name=f"hTd_{l}", bufs=2)
                nc.vector.tensor_copy(hT_new[:], psT[:])
                hT_dec[l] = hT_new[:]
                h_below = hT_new[:]

            # linear head: out = h3 @ lin_W.T + lin_b  -> (16,1)
            pso = pspool.tile([BC, 16], F32, tag="psS", name="ps_o", bufs=2)
            nc.tensor.matmul(pso[:], ones1[:], linb[:], start=True, stop=False)
            for c in range(CH):
                nc.tensor.matmul(pso[:], hT_dec[3][:, 16 * c:16 * (c + 1)], linw[:, 16 * c:16 * (c + 1)],
                                 start=False, stop=(c == CH - 1))
            # next input columns: [o0, x0-o0, x1-(x0-o0), 1]
            xin_new = sbuf.tile([BC, 16], F32R, tag="xin", name="xin", bufs=2)
            nc.vector.tensor_copy(xin_new[:, 0:1], pso[:, 0:1])
            nc.vector.tensor_tensor(xin_new[:, 1:2], xin[:, 0:1], xin_new[:, 0:1], mybir.AluOpType.subtract)
            nc.vector.tensor_tensor(xin_new[:, 2:3], xin[:, 1:2], xin_new[:, 1:2], mybir.AluOpType.subtract)
            nc.vector.tensor_copy(xin_new[:, 3:16], onescol[:])
            nc.vector.tensor_copy(outcol[:, t:t + 1], pso[:, 0:1])
            xin = xin_new

        nc.sync.dma_start(out_e[:], outcol[:])

    nc.compile()
    _prog_cache["nc"] = nc
    return nc


def kernel(x, y, enc_Wih0, enc_Whh0, enc_Wih1, enc_Whh1, enc_bih, enc_bhh,
           dec_Wih0, dec_Wihr, dec_Whh, dec_bih, dec_bhh, lin_W, lin_b,
           target_len, teacher_forcing_ratio):
    x = np.asarray(x, np.float32)
    f = np.float32

    def chunksT(W):  # (H,K) -> (K//128, 128, H) chunks of W.T
        WT = np.ascontiguousarray(W.T.astype(f))
        return WT.reshape(WT.shape[0] // 128, 128, WT.shape[1])

    whh0 = np.stack([chunksT(np.asarray(enc_Whh0)[d]) for d in range(2)])
    whh1 = np.stack([chunksT(np.asarray(enc_Whh1)[d]) for d in range(2)])
    wih1 = np.stack([chunksT(np.asarray(enc_Wih1)[d]) for d in range(2)])
    dwhh = np.stack([chunksT(np.asarray(dec_Whh)[l]) for l in range(4)])
    dwihr = np.stack([chunksT(np.asarray(dec_Wihr)[l]) for l in range(3)])
    bias1 = np.stack([(np.asarray(enc_bih)[1, d] + np.asarray(enc_bhh)[1, d]).astype(f)[None, :]
                      for d in range(2)])
    dbias = np.stack([(np.asarray(dec_bih)[l] + np.asarray(dec_bhh)[l]).astype(f)[None, :]
                      for l in range(1, 4)])
    din0w = np.zeros((16, H), f)
    din0w[:3] = np.asarray(dec_Wih0, f).T
    din0w[3] = (np.asarray(dec_bih)[0] + np.asarray(dec_bhh)[0]).astype(f)
    linw = np.zeros((CH, 128, 16), f)
    linw[:, :, 0] = np.asarray(lin_W, f).T.reshape(CH, 128)
    linb = np.zeros((1, 16), f)
    linb[0, 0] = np.asarray(lin_b, f).reshape(())
    ident = np.eye(16, dtype=f)
    ones1 = np.ones((1, 16), f)

    nc = _build_program()
    in_maps = []
    for c in range(NC):
        xc = x[c * BC:(c + 1) * BC]  # (16, T, 3)
        xs0 = np.empty((2, T, BC, H), f)
        for d in range(2):
            W = np.asarray(enc_Wih0)[d].astype(f)
            b = (np.asarray(enc_bih)[0, d] + np.asarray(enc_bhh)[0, d]).astype(f)
            proj = np.einsum('bti,hi->tbh', xc, W) + b  # (T, 16, H)
            xs0[d] = proj if d == 0 else proj[::-1]
        dinit = np.zeros((BC, 16), f)
        dinit[:, :3] = xc[:, -1, :]
        dinit[:, 3] = 1.0
        in_maps.append({
            "xs0": xs0, "whh0": whh0, "whh1": whh1, "wih1": wih1, "bias1": bias1,
            "dwhh": dwhh, "dwihr": dwihr, "dbias": dbias, "din0w": din0w,
            "linw": linw, "linb": linb, "ident": ident, "ones1": ones1, "dinit": dinit,
        })
    res = run_bass_kernel_spmd(nc, in_maps, list(range(NC)))
    out = np.concatenate([res.results[c]["out"] for c in range(NC)], 0)
    return out.reshape(B, TGT, 1).astype(np.float32)


# revision 7
# speedup vs baseline: 1.1536x; 1.1536x over previous
"""BiRNN encoder-decoder Trainium2 kernel.

Strategy: data-parallel over batch (8 cores x 16 rows). All matmuls in
float32r (tf32-like, 1 cyc/row at moving>=256). Recurrences use "layout B":
psum out = (b, H) with state-transpose (hT) as the stationary operand,
weights as the 512-col moving operand; hT is regenerated each step via 4
PE-transposes + one DVE copy. Layer-0 input projections (+all biases) are
precomputed on the host and DMA-prefetched per step; layer-1's input
projection is fused into its recurrence as 8 extra moving-weight matmuls
against the stored layer-0 hT chunks.
"""
import numpy as np
from contextlib import ExitStack

import concourse.bacc as bacc
import concourse.tile as tile
from concourse import mybir
from concourse.bass_utils import run_bass_kernel_spmd

B, T, IN, H, TGT = 128, 128, 3, 512, 32
NC = 8
BC = B // NC  # 16 batch rows per core
F32R = mybir.dt.float32r
F32 = mybir.dt.float32
CH = H // 128  # 4 chunks of the hidden dim

_prog_cache = {}


def _build_program():
    if "nc" in _prog_cache:
        return _prog_cache["nc"]
    nc = bacc.Bacc("TRN2")
    dp = nc.declare_dram_parameter

    xs0_e = dp("xs0", [2, T, BC, H], F32R, isOutput=False)          # per-dir l0 x-proj (+biases), bwd time-reversed
    whh0_e = dp("whh0", [2, CH, 128, H], F32R, isOutput=False)      # enc l0 Whh.T chunks
    whh1_e = dp("whh1", [2, CH, 128, H], F32R, isOutput=False)      # enc l1 Whh.T chunks
    wih1_e = dp("wih1", [2, 2 * CH, 128, H], F32R, isOutput=False)  # enc l1 Wih.T chunks (k<4 fwd-half, k>=4 bwd-half)
    bias1_e = dp("bias1", [2, 1, H], F32R, isOutput=False)          # enc l1 bih+bhh rows
    dwhh_e = dp("dwhh", [4, CH, 128, H], F32R, isOutput=False)      # dec Whh.T chunks
    dwihr_e = dp("dwihr", [3, CH, 128, H], F32R, isOutput=False)    # dec Wihr.T chunks
    dbias_e = dp("dbias", [3, 1, H], F32R, isOutput=False)          # dec l1..3 bias rows
    din0w_e = dp("din0w", [16, H], F32R, isOutput=False)             # [dec_Wih0.T(3,H); bias0(1,H)]
    linw_e = dp("linw", [CH, 128, 16], F32R, isOutput=False)         # lin_W.T chunks
    linb_e = dp("linb", [1, 16], F32R, isOutput=False)
    ident_e = dp("ident", [16, 16], F32R, isOutput=False)
    ones1_e = dp("ones1", [1, 16], F32R, isOutput=False)
    dinit_e = dp("dinit", [BC, 16], F32R, isOutput=False)            # [x_last(16,3), ones]
    out_e = dp("out", [BC, TGT], F32, isOutput=True)

    with tile.TileContext(nc) as tc, ExitStack() as ctx:
        wpool = ctx.enter_context(tc.tile_pool(name="w", bufs=1))
        hts = ctx.enter_context(tc.tile_pool(name="hts", bufs=1))
        sbuf = ctx.enter_context(tc.tile_pool(name="sb", bufs=1))
        pspool = ctx.enter_context(tc.tile_pool(name="ps", bufs=1, space="PSUM"))

        # ---- weights / constants into SBUF ----
        whh0 = wpool.tile([128, 2 * CH * H], F32R)
        whh1 = wpool.tile([128, 2 * CH * H], F32R)
        wih1 = wpool.tile([128, 2 * 2 * CH * H], F32R)
        dwhh = wpool.tile([128, 4 * CH * H], F32R)
        dwihr = wpool.tile([128, 3 * CH * H], F32R)
        for d in range(2):
            for c in range(CH):
                nc.gpsimd.dma_start(whh0[:, (d * CH + c) * H:(d * CH + c + 1) * H], whh0_e[d, c])
                nc.gpsimd.dma_start(whh1[:, (d * CH + c) * H:(d * CH + c + 1) * H], whh1_e[d, c])
            for k in range(2 * CH):
                nc.gpsimd.dma_start(wih1[:, (d * 2 * CH + k) * H:(d * 2 * CH + k + 1) * H], wih1_e[d, k])
        for l in range(4):
            for c in range(CH):
                nc.gpsimd.dma_start(dwhh[:, (l * CH + c) * H:(l * CH + c + 1) * H], dwhh_e[l, c])
        for l in range(3):
            for c in range(CH):
                nc.gpsimd.dma_start(dwihr[:, (l * CH + c) * H:(l * CH + c + 1) * H], dwihr_e[l, c])
        linw = wpool.tile([128, CH * 16], F32R)
        for c in range(CH):
            nc.gpsimd.dma_start(linw[:, 16 * c:16 * (c + 1)], linw_e[c])
        bias1 = wpool.tile([1, 2 * H], F32R)
        dbias = wpool.tile([1, 3 * H], F32R)
        for d in range(2):
            nc.gpsimd.dma_start(bias1[:, d * H:(d + 1) * H], bias1_e[d])
        for l in range(3):
            nc.gpsimd.dma_start(dbias[:, l * H:(l + 1) * H], dbias_e[l])
        din0w = wpool.tile([16, H], F32R)
        nc.gpsimd.dma_start(din0w[:], din0w_e[:])
        linb = wpool.tile([1, 16], F32R)
        nc.gpsimd.dma_start(linb[:], linb_e[:])
        ident = wpool.tile([16, 16], F32R)
        nc.gpsimd.dma_start(ident[:], ident_e[:])
        ones1 = wpool.tile([1, 16], F32R)
        nc.gpsimd.dma_start(ones1[:], ones1_e[:])
        onescol = wpool.tile([BC, 13], F32R)
        nc.gpsimd.dma_start(onescol[:], dinit_e[:, 3:16])

        # per-t l0 hidden-state transposes, both directions (bwd in its own step order)
        hT0 = {d: hts.tile([128, T * 4 * BC], F32R, tag=f"hT0_{d}", name=f"hT0_{d}") for d in range(2)}

        def recur_step(ps_tag, h_tag, whh_tile, whh_off, hT_cur, hT_dst, inject):
            """One RNN step: psum = inject + h @ Whh.T; tanh; rebuild hT."""
            ps = pspool.tile([BC, H], F32, tag=ps_tag, name=ps_tag, bufs=2)
            inject(ps)
            for c in range(CH):
                nc.tensor.matmul(ps[:], hT_cur[:, 16 * c:16 * (c + 1)],
                                 whh_tile[:, whh_off + c * H: whh_off + (c + 1) * H],
                                 start=False, stop=(c == CH - 1))
            h = sbuf.tile([BC, H], F32R, tag="h", name="h", bufs=2)
            nc.scalar.activation(h[:], ps[:], mybir.ActivationFunctionType.Tanh)
            psT = pspool.tile([128, 4 * BC], F32R, tag="psT", name="psT", bufs=2)
            for c in range(CH):
                nc.tensor.transpose(psT[:, 16 * c:16 * (c + 1)], h[:, 128 * c:128 * (c + 1)], ident[:])
            nc.vector.tensor_copy(hT_dst[:], psT[:])
            return h

        # ---- encoder layer 0 (fwd chain d=0, bwd chain d=1, interleaved) ----
        hT_cur = {}
        for t in range(T):
            for d in range(2):
                xs = sbuf.tile([BC, H], F32R, tag="xs", name="xs", bufs=2)
                nc.sync.dma_start(xs[:], xs0_e[d, t])

                def inject(ps, xs=xs):
                    nc.tensor.matmul(ps[:], ident[:], xs[:], start=True, stop=False)
                dst = hT0[d][:, t * 4 * BC:(t + 1) * 4 * BC]
                if t == 0:
                    # h0 = 0: first Whh matmul contributes zeros via zeroed hT? No -
                    # instead: psum = inject only, skip Whh matmuls entirely.
                    ps = pspool.tile([BC, H], F32, tag="psA" if d == 0 else "psB", name="ps0", bufs=2)
                    nc.tensor.matmul(ps[:], ident[:], xs[:], start=True, stop=True)
                    h = sbuf.tile([BC, H], F32R, tag="h", name="h", bufs=2)
                    nc.scalar.activation(h[:], ps[:], mybir.ActivationFunctionType.Tanh)
                    psT = pspool.tile([128, 4 * BC], F32R, tag="psT", name="psT", bufs=2)
                    for c in range(CH):
                        nc.tensor.transpose(psT[:, 16 * c:16 * (c + 1)], h[:, 128 * c:128 * (c + 1)], ident[:])
                    nc.vector.tensor_copy(dst, psT[:])
                else:
                    recur_step("psA" if d == 0 else "psB", f"h0_{d}", whh0, d * CH * H, hT_cur[d], dst, inject)
                hT_cur[d] = dst

        # ---- encoder layer 1 (input = stored l0 hT of both dirs, fused proj) ----
        hT1 = {}
        for t in range(T):
            for d in range(2):
                # kernel-step t of chain d corresponds to real time rt:
                # d=0: rt=t -> fwd l0 slot t, bwd l0 slot T-1-t
                # d=1: rt=T-1-t -> fwd l0 slot T-1-t, bwd l0 slot t
                f_slot = t if d == 0 else T - 1 - t
                b_slot = T - 1 - t if d == 0 else t

                def inject(ps, d=d, f_slot=f_slot, b_slot=b_slot):
                    nc.tensor.matmul(ps[:], ones1[:], bias1[:, d * H:(d + 1) * H], start=True, stop=False)
                    for k in range(2 * CH):
                        src = hT0[0] if k < CH else hT0[1]
                        slot = f_slot if k < CH else b_slot
                        cc = k % CH
                        nc.tensor.matmul(
                            ps[:], src[:, slot * 4 * BC + 16 * cc: slot * 4 * BC + 16 * (cc + 1)],
                            wih1[:, (d * 2 * CH + k) * H:(d * 2 * CH + k + 1) * H],
                            start=False, stop=False)
                dst = hts.tile([128, 4 * BC], F32R, tag=f"hT1_{d}", name=f"hT1_{d}", bufs=2)
                if t == 0:
                    # no Whh term at t=0 (h0 = 0): close the group on the last inject mm
                    ps = pspool.tile([BC, H], F32, tag="psA" if d == 0 else "psB", name="ps1", bufs=2)
                    nc.tensor.matmul(ps[:], ones1[:], bias1[:, d * H:(d + 1) * H], start=True, stop=False)
                    for k in range(2 * CH):
                        src = hT0[0] if k < CH else hT0[1]
                        slot = f_slot if k < CH else b_slot
                        cc = k % CH
                        nc.tensor.matmul(
                            ps[:], src[:, slot * 4 * BC + 16 * cc: slot * 4 * BC + 16 * (cc + 1)],
                            wih1[:, (d * 2 * CH + k) * H:(d * 2 * CH + k + 1) * H],
                            start=False, stop=(k == 2 * CH - 1))
                    h = sbuf.tile([BC, H], F32R, tag="h", name="h", bufs=2)
                    nc.scalar.activation(h[:], ps[:], mybir.ActivationFunctionType.Tanh)
                    psT = pspool.tile([128, 4 * BC], F32R, tag="psT", name="psT", bufs=2)
                    for c in range(CH):
                        nc.tensor.transpose(psT[:, 16 * c:16 * (c + 1)], h[:, 128 * c:128 * (c + 1)], ident[:])
                    nc.vector.tensor_copy(dst[:], psT[:])
                else:
                    recur_step("psA" if d == 0 else "psB", f"h1_{d}", whh1, d * CH * H, hT1[d], dst[:], inject)
                hT1[d] = dst[:]

        # ---- decoder: 4-layer stack, 32 autoregressive steps ----
        hT_dec = {0: hT0[0][:, (T - 1) * 4 * BC: T * 4 * BC],  # hf0
                  1: hT0[1][:, (T - 1) * 4 * BC: T * 4 * BC],  # hb0 (its last kernel step = real t0)
                  2: hT1[0], 3: hT1[1]}
        xin = sbuf.tile([BC, 16], F32R, tag="xin", name="xin", bufs=2)
        nc.sync.dma_start(xin[:], dinit_e[:])
        outcol = sbuf.tile([BC, TGT], F32, tag="outcol", name="outcol")

        for t in range(TGT):
            # transpose current input cols (16,4) -> (4,16) for the l0 inject
            psx = pspool.tile([16, 16], F32R, tag="psS", name="psxT", bufs=2)
            nc.tensor.transpose(psx[:], xin[:], ident[:])
            xinT = sbuf.tile([16, 16], F32R, tag="xinT", name="xinT")
            nc.vector.tensor_copy(xinT[:], psx[:])

            h_below = None
            for l in range(4):
                ps = pspool.tile([BC, H], F32, tag="psA", name="ps_dec", bufs=2)
                if l == 0:
                    nc.tensor.matmul(ps[:], xinT[:], din0w[:], start=True, stop=False)
                else:
                    nc.tensor.matmul(ps[:], ones1[:], dbias[:, (l - 1) * H:l * H], start=True, stop=False)
                    for c in range(CH):
                        nc.tensor.matmul(ps[:], h_below[:, 16 * c:16 * (c + 1)],
                                         dwihr[:, ((l - 1) * CH + c) * H:((l - 1) * CH + c + 1) * H],
                                         start=False, stop=False)
                for c in range(CH):
                    nc.tensor.matmul(ps[:], hT_dec[l][:, 16 * c:16 * (c + 1)],
                                     dwhh[:, (l * CH + c) * H:(l * CH + c + 1) * H],
                                     start=False, stop=(c == CH - 1))
                h = sbuf.tile([BC, H], F32R, tag="h", name="h", bufs=2)
                nc.scalar.activation(h[:], ps[:], mybir.ActivationFunctionType.Tanh)
                psT = pspool.tile([128, 4 * BC], F32R, tag="psT", name="psT", bufs=2)
                for c in range(CH):
                    nc.tensor.transpose(psT[:, 16 * c:16 * (c + 1)], h[:, 128 * c:128 * (c + 1)], ident[:])
                hT_new = hts.tile([128, 4 * BC], F32R, tag=f"hTd_{l}", name=f"hTd_{l}", bufs=2)
                nc.vector.tensor_copy(hT_new[:], psT[:])
                hT_dec[l] = hT_new[:]
                h_below = hT_new[:]

            # linear head: out = h3 @ lin_W.T + lin_b  -> (16,1)
            pso = pspool.tile([BC, 16], F32, tag="psS", name="ps_o", bufs=2)
            nc.tensor.matmul(pso[:], ones1[:], linb[:], start=True, stop=False)
            for c in range(CH):
                nc.tensor.matmul(pso[:], hT_dec[3][:, 16 * c:16 * (c + 1)], linw[:, 16 * c:16 * (c + 1)],
                                 start=False, stop=(c == CH - 1))
            # next input columns: [o0, x0-o0, x1-(x0-o0), 1]
            xin_new = sbuf.tile([BC, 16], F32R, tag="xin", name="xin", bufs=2)
            nc.vector.tensor_copy(xin_new[:, 0:1], pso[:, 0:1])
            nc.vector.tensor_tensor(xin_new[:, 1:2], xin[:, 0:1], xin_new[:, 0:1], mybir.AluOpType.subtract)
            nc.vector.tensor_tensor(xin_new[:, 2:3], xin[:, 1:2], xin_new[:, 1:2], mybir.AluOpType.subtract)
            nc.vector.tensor_copy(xin_new[:, 3:16], onescol[:])
            nc.vector.tensor_copy(outcol[:, t:t + 1], pso[:, 0:1])
            xin = xin_new

        nc.sync.dma_start(out_e[:], outcol[:])

    nc.compile()
    _prog_cache["nc"] = nc
    return nc


def kernel(x, y, enc_Wih0, enc_Whh0, enc_Wih1, enc_Whh1, enc_bih, enc_bhh,
           dec_Wih0, dec_Wihr, dec_Whh, dec_bih, dec_bhh, lin_W, lin_b,
           target_len, teacher_forcing_ratio):
    x = np.asarray(x, np.float32)
    f = np.float32

    def chunksT(W):  # (H,K) -> (K//128, 128, H) chunks of W.T
        WT = np.ascontiguousarray(W.T.astype(f))
        return WT.reshape(WT.shape[0] // 128, 128, WT.shape[1])

    whh0 = np.stack([chunksT(np.asarray(enc_Whh0)[d]) for d in range(2)])
    whh1 = np.stack([chunksT(np.asarray(enc_Whh1)[d]) for d in range(2)])
    wih1 = np.stack([chunksT(np.asarray(enc_Wih1)[d]) for d in range(2)])
    dwhh = np.stack([chunksT(np.asarray(dec_Whh)[l]) for l in range(4)])
    dwihr = np.stack([chunksT(np.asarray(dec_Wihr)[l]) for l in range(3)])
    bias1 = np.stack([(np.asarray(enc_bih)[1, d] + np.asarray(enc_bhh)[1, d]).astype(f)[None, :]
                      for d in range(2)])
    dbias = np.stack([(np.asarray(dec_bih)[l] + np.asarray(dec_bhh)[l]).astype(f)[None, :]
                      for l in range(1, 4)])
    din0w = np.zeros((16, H), f)
    din0w[:3] = np.asarray(dec_Wih0, f).T
    din0w[3] = (np.asarray(dec_bih)[0] + np.asarray(dec_bhh)[0]).astype(f)
    linw = np.zeros((CH, 128, 16), f)
    linw[:, :, 0] = np.asarray(lin_W, f).T.reshape(CH, 128)
    linb = np.zeros((1, 16), f)
    linb[0, 0] = np.asarray(lin_b, f).reshape(())
    ident = np.eye(16, dtype=f)
    ones1 = np.ones((1, 16), f)

    nc = _build_program()
    in_maps = []
    for c in range(NC):
        xc = x[c * BC:(c + 1) * BC]  # (16, T, 3)
        xs0 = np.empty((2, T, BC, H), f)
        for d in range(2):
            W = np.asarray(enc_Wih0)[d].astype(f)
            b = (np.asarray(enc_bih)[0, d] + np.asarray(enc_bhh)[0, d]).astype(f)
            proj = np.einsum('bti,hi->tbh', xc, W) + b  # (T, 16, H)
            xs0[d] = proj if d == 0 else proj[::-1]
        dinit = np.zeros((BC, 16), f)
        dinit[:, :3] = xc[:, -1, :]
        dinit[:, 3] = 1.0
        in_maps.append({
            "xs0": xs0, "whh0": whh0, "whh1": whh1, "wih1": wih1, "bias1": bias1,
            "dwhh": dwhh, "dwihr": dwihr, "dbias": dbias, "din0w": din0w,
            "linw": linw, "linb": linb, "ident": ident, "ones1": ones1, "dinit": dinit,
        })
    res = run_bass_kernel_spmd(nc, in_maps, list(range(NC)))
    out = np.concatenate([res.results[c]["out"] for c in range(NC)], 0)
    return out.reshape(B, TGT, 1).astype(np.float32)
